# revision 14
# baseline (speedup 1.0000x reference)
"""AddShift_mp_linear_module on 8 TRN2 NeuronCores — v4 (column-tiled PE,
consolidated DMA).

Strategy (channel-block sharding, no collectives):
  - 96 output-channel blocks (11 input channels each) -> 12 blocks/core,
    processed as 6 block-pairs.
  - Per block, three dense contractions, all with M=56 output columns:
      V:  out_v[h, (b,w)]  over (k,h') rows   (h-major x)
      H:  out_hT[w, (b,h)] over (k,w') rows   (w-major x)
      I:  out_i[h, (b,w)]  over gathered identity rows
    TWO chains run CONCURRENTLY on the PE via column tiling
    (tile_position (0,0) / (0,64)); measured pair span == single span.
  - Contraction rows per block: 660 = 5x128 + 20. K=128 chunks keep all
    16 SDMA engines and all PE rows busy; the 20-row tails of all blocks
    ship as one small transfer.
  - DMA consolidation: ONE 1.38 MB transfer per pair (x both orientations
    + identity gather), 2 operator transfers, 1 tail transfer, 1 output
    transfer per pair. ~15 DMAs total, ~10.8 MB/core (the roofline).
  - Precision: fp8 e3m4 in, bf16 out (rel_err 1.85e-2 < 2e-2).
"""

import numpy as np
import ml_dtypes

# architecture constants (match reference init_kwargs)
B = 8
C_OUT = 96
NK = 11
G = 4
C_IN = C_OUT * NK          # 1056
HOUT = WOUT = 56
HIN = WIN = 60
EP = 2                     # extra pad
N_CORES = 8
BPC = C_OUT // N_CORES     # blocks per core = 12
CPC = BPC * NK             # channels per core = 132
NPAIR = BPC // 2           # block pairs per core = 6
KROWS = NK * HIN           # 660 contraction rows per block (V/H)
KM = 128                   # main chunk rows
NJM = 5                    # main chunks (5 x 128 = 640)
KT = KROWS - NJM * KM      # tail chunk rows = 20
NFREE = B * WOUT           # 448 matmul free dim
N_WARM = 16                # PE warmup matmuls while first DMAs fly
OPW = NJM * 112 + 2 * 56   # operator cols per block = 672
XTW = 112 + 2 * NFREE      # tail cols per block = 1008

F8 = ml_dtypes.float8_e3m4

_CACHE = {}


def _build_vh_operators(w1, w2, pad_hv):
    """Dense V/H operators per block: (C_OUT, 660, 56) fp32 each.
    Row r = k*60 + spatial_in for channel c = co*11 + k."""
    w1r = np.asarray(w1, np.float32).reshape(G, C_IN)
    w2r = np.asarray(w2, np.float32).reshape(G, C_IN)
    pad = np.asarray(pad_hv, np.int64)            # (C_IN, 2G)
    opv = np.zeros((C_OUT, KROWS, 56), np.float32)
    oph = np.zeros((C_OUT, KROWS, 56), np.float32)
    c_all = np.arange(C_IN)
    co_all = c_all // NK
    k_all = c_all % NK
    pos = np.arange(HOUT)
    for g in range(G):
        win = pos[None, :] + EP + pad[:, g][:, None]        # (C_IN, 56)
        ok = (win >= 0) & (win < WIN)
        cc, oo = np.nonzero(ok)
        np.add.at(oph, (co_all[cc], k_all[cc] * HIN + win[cc, oo], oo), w1r[g, cc])
        hin = pos[None, :] + EP + pad[:, G + g][:, None]
        ok = (hin >= 0) & (hin < HIN)
        cc, oo = np.nonzero(ok)
        np.add.at(opv, (co_all[cc], k_all[cc] * HIN + hin[cc, oo], oo), w2r[g, cc])
    return opv, oph


def _identity_slots(w3, idx_identit):
    """Per block: up to 4 (k, coeff) identity terms, padded to 4 slots."""
    w3r = np.asarray(w3, np.float32).reshape(G, C_OUT)
    idx = np.asarray(idx_identit, np.int64)       # (C_OUT, G)
    k_sel = idx - np.arange(C_OUT)[:, None] * NK
    assert np.all((k_sel >= 0) & (k_sel < NK))
    u = np.zeros((C_OUT, NK), np.float32)
    for g in range(G):
        np.add.at(u, (np.arange(C_OUT), k_sel[:, g]), w3r[g])
    ks = np.zeros((C_OUT, 4), np.int64)
    cf = np.zeros((C_OUT, 4), np.float32)
    for co in range(C_OUT):
        nz = np.nonzero(u[co])[0]
        ks[co, :len(nz)] = nz
        cf[co, :len(nz)] = u[co, nz]
    return ks, cf


def _build_nc():
    import concourse.bacc as bacc
    import concourse.tile as tile
    import concourse.bass as bass
    import concourse.mybir as mybir
    from contextlib import ExitStack

    f32 = mybir.dt.float32
    f8 = mybir.dt.float8e3
    bf16 = mybir.dt.bfloat16

    nc = bacc.Bacc(None, target_bir_lowering=False)
    # x per pair: [p, slot, blk, chunk, n]; slot 0 = h-major (V) / identity
    # chunk0, slot 1 = w-major (H) / identity chunk1; chunk 5 is identity.
    xall_d = nc.declare_dram_parameter(
        "xall", [BPC, KM, 2, NJM + 1, NFREE], f8, isOutput=False)
    # operators per block: [p, bi, 672]: 5x112 V|H chunks then 2x56 identity
    opall_d = nc.declare_dram_parameter(
        "opall", [KM, BPC, OPW], f8, isOutput=False)
    # tails: [p(20), bi, 1008]: 112 V|H operator, then x tails (o=0, o=1)
    opxt_d = nc.declare_dram_parameter(
        "opxt", [KT, BPC, XTW], f8, isOutput=False)
    # output: per pair [120, 3, 448] (cols: blk_e VH | blk_o VH | identity)
    out_d = nc.declare_dram_parameter(
        "out", [NPAIR, 120, 3, NFREE], bf16, isOutput=True)

    with tile.TileContext(nc) as tc, ExitStack() as ctx:
        xpool = ctx.enter_context(tc.tile_pool(name="xp", bufs=1))
        oppool = ctx.enter_context(tc.tile_pool(name="opp", bufs=1))
        spool = ctx.enter_context(tc.tile_pool(name="stg", bufs=1))
        wpool = ctx.enter_context(tc.tile_pool(name="wp", bufs=1))
        psum_pool = ctx.enter_context(
            tc.tile_pool(name="psum", bufs=2, space=bass.MemorySpace.PSUM))
        wppool = ctx.enter_context(
            tc.tile_pool(name="wpp", bufs=1, space=bass.MemorySpace.PSUM))

        # ---- ring S (sync): tails+operators interleaved with per-block x;
        # ring A (scalar) carries only outputs ----
        op_t = oppool.tile([KM, BPC, OPW], f8, tag="opall")
        xt_t = oppool.tile([KT, BPC, XTW], f8, tag="opxt")
        nc.sync.dma_start(xt_t[:], opxt_d[:])
        x_ts = [None] * BPC

        def load_x(bi):
            x_t = xpool.tile([KM, 2, NJM + 1, NFREE], f8, tag=f"x{bi}",
                             name=f"x{bi}")
            if bi == BPC - 1:
                nc.sync.dma_start(x_t[:, :, 0:3, :], xall_d[bi][:, :, 0:3])
                nc.sync.dma_start(x_t[:, :, 3:NJM + 1, :],
                                  xall_d[bi][:, :, 3:NJM + 1])
            else:
                nc.sync.dma_start(x_t[:], xall_d[bi])
            x_ts[bi] = x_t

        for q in range(NPAIR):
            nc.sync.dma_start(op_t[:, 2 * q:2 * q + 2],
                              opall_d[:, 2 * q:2 * q + 2])
            load_x(2 * q)
            load_x(2 * q + 1)

        # ---- PE warmup on memset tiles (no DMA dependency) ----
        warm = wpool.tile([KM, NFREE], f8, tag="warm")
        wst = wpool.tile([KM, 56], f8, tag="wst")
        nc.vector.memset(warm[:], 0)
        nc.vector.memset(wst[:], 0)
        pw = wppool.tile([128, NFREE], f32, tag="pw")
        for w in range(N_WARM):
            pos = (0, 0) if w % 2 == 0 else (0, 64)
            dst = pw[0:56] if w % 2 == 0 else pw[64:120]
            nc.tensor.matmul(dst, wst[:], warm[:], start=True, stop=True,
                             tile_position=pos)

        # ---- main: 6 block pairs ----
        out_stgs = []
        for q in range(NPAIR):
            pvh = [psum_pool.tile([128, NFREE], f32, tag="pe", name=f"pe{q}"),
                   psum_pool.tile([128, NFREE], f32, tag="po", name=f"po{q}")]
            pi = psum_pool.tile([128, NFREE], f32, tag="pi", name=f"pi{q}")
            for b in (0, 1):
                pt = pvh[b]
                bi = 2 * q + b
                x_t = x_ts[bi]
                for j in range(NJM):
                    nc.tensor.matmul(pt[0:56], op_t[:, bi, j * 112:j * 112 + 56],
                                     x_t[:, 0, j, :],
                                     start=(j == 0), stop=False,
                                     tile_position=(0, 0))
                    nc.tensor.matmul(pt[64:120],
                                     op_t[:, bi, j * 112 + 56:(j + 1) * 112],
                                     x_t[:, 1, j, :],
                                     start=(j == 0), stop=False,
                                     tile_position=(0, 64))
                nc.tensor.matmul(pt[0:56], xt_t[:, bi, 0:56],
                                 xt_t[:, bi, 112:112 + NFREE],
                                 start=False, stop=True,
                                 tile_position=(0, 0))
                nc.tensor.matmul(pt[64:120], xt_t[:, bi, 56:112],
                                 xt_t[:, bi, 112 + NFREE:XTW],
                                 start=False, stop=True,
                                 tile_position=(0, 64))
            for c in range(2):
                kk = 128 if c == 0 else 96  # identity rows: 224 = 128 + 96
                o0 = NJM * 112 + c * 56
                nc.tensor.matmul(pi[0:56], op_t[0:kk, 2 * q, o0:o0 + 56],
                                 x_ts[2 * q][0:kk, c, NJM, :],
                                 start=(c == 0), stop=(c == 1),
                                 tile_position=(0, 0))
                nc.tensor.matmul(pi[64:120], op_t[0:kk, 2 * q + 1, o0:o0 + 56],
                                 x_ts[2 * q + 1][0:kk, c, NJM, :],
                                 start=(c == 0), stop=(c == 1),
                                 tile_position=(0, 64))
            # drain psums -> bf16 staging -> one DMA per pair (ring A)
            stg = spool.tile([120, 3, NFREE], bf16, tag=f"stg{q}", name=f"stg{q}")
            nc.scalar.copy(stg[:, 0, :], pvh[0][0:120])
            nc.vector.tensor_copy(stg[:, 1, :], pvh[1][0:120])
            nc.vector.tensor_copy(stg[0:56, 2, :], pi[0:56])
            nc.scalar.copy(stg[64:120, 2, :], pi[64:120])
            out_stgs.append(stg)
        for q, stg in enumerate(out_stgs):
            if q == NPAIR - 1:
                nc.sync.dma_start(out_d[q, :, 0:2], stg[:, 0:2, :])
                nc.sync.dma_start(out_d[q, :, 2], stg[:, 2, :])
            else:
                nc.sync.dma_start(out_d[q], stg[:])
    nc.finalize()
    return nc


def prepare_inputs(x, w1, w2, w3, pad_hv, idx_identit):
    """Host-side shard prep. Returns in_maps (list of 8 dicts)."""
    x = np.asarray(x)
    xq = x.astype(F8)                                     # (B, C, 60, 60)
    opv, oph = _build_vh_operators(w1, w2, pad_hv)        # (96, 660, 56) f32
    ks, cf = _identity_slots(w3, idx_identit)             # (96,4) each
    eye = np.eye(56, dtype=np.float32)

    in_maps = []
    for i in range(N_CORES):
        blocks = np.arange(i * BPC, (i + 1) * BPC)
        csl = slice(i * CPC, (i + 1) * CPC)
        # h-major rows (k,h'), cols (b,w); w-major rows (k,w'), cols (b,h)
        ch = xq[:, csl, :, EP:EP + WOUT]                   # (8, 132, 60, 56)
        ch = ch.transpose(1, 2, 0, 3).reshape(BPC, KROWS, NFREE)
        cw = xq[:, csl, EP:EP + HOUT, :]                   # (8, 132, 56, 60)
        cw = cw.transpose(1, 3, 0, 2).reshape(BPC, KROWS, NFREE)
        xo = np.stack([ch, cw], axis=1)                    # (12, 2, 660, 448)
        # identity gather rows (slot, h'): 224 per block -> chunks 128 + 96
        chan = (blocks[:, None] * NK + ks[blocks]).astype(np.int64)  # (12,4)
        xi_raw = xq[:, chan, EP:EP + HOUT, EP:EP + WOUT]   # (8, 12, 4, 56, 56)
        xi_raw = xi_raw.transpose(1, 2, 3, 0, 4).reshape(BPC, 224, NFREE)
        # xall: [bi, p, slot, chunk(6), n]
        xall = np.zeros((BPC, KM, 2, NJM + 1, NFREE), F8)
        main = xo[:, :, :NJM * KM].reshape(BPC, 2, NJM, KM, NFREE)
        xall[:, :, :, :NJM] = main.transpose(0, 3, 1, 2, 4)
        xi_chunks = np.zeros((BPC, 2, KM, NFREE), F8)
        xi_chunks[:, 0] = xi_raw[:, :KM]
        xi_chunks[:, 1, :224 - KM] = xi_raw[:, KM:]
        xall[:, :, :, NJM] = xi_chunks.transpose(0, 2, 1, 3)
        # opall: [p, bi, 672]
        opvh = np.concatenate([opv[blocks], oph[blocks]], axis=2)  # (12,660,112)
        opm = (opvh[:, :NJM * KM].reshape(BPC, NJM, KM, 112)
               .transpose(2, 0, 1, 3).reshape(KM, BPC, NJM * 112))
        bands = (cf[blocks][:, :, None, None] * eye).reshape(BPC, 224, 56)
        opi_pad = np.zeros((BPC, 2, KM, 56), np.float32)
        opi_pad[:, 0] = bands[:, :KM]
        opi_pad[:, 1, :224 - KM] = bands[:, KM:]
        opi = opi_pad.transpose(2, 0, 1, 3).reshape(KM, BPC, 112)
        opall = np.ascontiguousarray(
            np.concatenate([opm, opi], axis=2)).astype(F8)  # (128, 12, 672)
        # opxt: [p(20), bi, 1008]: operator tail 112 | x tail o=0 | o=1
        optail = opvh[:, NJM * KM:].transpose(1, 0, 2)      # (20, 12, 112)
        xtail = (xo[:, :, NJM * KM:]                        # (12, 2, 20, 448)
                 .transpose(2, 0, 1, 3).reshape(KT, BPC, 2 * NFREE))
        opxt = np.ascontiguousarray(
            np.concatenate([optail, xtail.reshape(KT, BPC, 2 * NFREE)], axis=2)
        ).astype(F8)                                        # (20, 12, 1008)
        in_maps.append({"xall": xall, "opall": opall, "opxt": opxt})
    return in_maps


def unshard(results):
    """-> (out_h, out_v, out_i) each (B, C_OUT, 56, 56) fp32."""
    o = np.stack([np.asarray(r["out"], np.float32) for r in results])
    # o: (8, NPAIR, 120, 3, 448); col 0 = blk_e, 1 = blk_o, 2 = identity
    vh = o[:, :, :, 0:2].transpose(0, 1, 3, 2, 4)  # (8, 6, 2, 120, 448)
    vh = vh.reshape(N_CORES, BPC, 120, NFREE)
    V = vh[:, :, 0:56].reshape(N_CORES, BPC, 56, B, WOUT)
    out_v = V.transpose(3, 0, 1, 2, 4).reshape(B, C_OUT, HOUT, WOUT)
    Hh = vh[:, :, 64:120].reshape(N_CORES, BPC, 56, B, HOUT)  # [.., w, b, h]
    out_h = Hh.transpose(3, 0, 1, 4, 2).reshape(B, C_OUT, HOUT, WOUT)
    ii = o[:, :, :, 2]                             # (8, 6, 120, 448)
    Ie = ii[:, :, 0:56].reshape(N_CORES, NPAIR, 56, B, WOUT)
    Io = ii[:, :, 64:120].reshape(N_CORES, NPAIR, 56, B, WOUT)
    I2 = np.stack([Ie, Io], axis=2)                # [core, pair, half, h, b, w]
    out_i = I2.transpose(4, 0, 1, 2, 3, 5).reshape(B, C_OUT, HOUT, WOUT)
    return out_h, out_v, out_i


def kernel(x, w1, w2, w3, pad_hv, idx_identit, b=B, hout=HOUT, wout=WOUT):
    from concourse.bass_utils import run_bass_kernel_spmd

    assert int(b) == B and int(hout) == HOUT and int(wout) == WOUT
    assert tuple(np.asarray(x).shape) == (B, C_IN, HIN, WIN)

    in_maps = prepare_inputs(x, w1, w2, w3, pad_hv, idx_identit)
    nc = _CACHE.get("nc")
    if nc is None:
        nc = _build_nc()
        _CACHE["nc"] = nc
    res = run_bass_kernel_spmd(nc, in_maps, core_ids=list(range(N_CORES)))
    return unshard(res.results)


# revision 15
# speedup vs baseline: 1.0575x; 1.0575x over previous
"""AddShift_mp_linear_module on 8 TRN2 NeuronCores — v4 (column-tiled PE,
consolidated DMA).

Strategy (channel-block sharding, no collectives):
  - 96 output-channel blocks (11 input channels each) -> 12 blocks/core,
    processed as 6 block-pairs.
  - Per block, three dense contractions, all with M=56 output columns:
      V:  out_v[h, (b,w)]  over (k,h') rows   (h-major x)
      H:  out_hT[w, (b,h)] over (k,w') rows   (w-major x)
      I:  out_i[h, (b,w)]  over gathered identity rows
    TWO chains run CONCURRENTLY on the PE via column tiling
    (tile_position (0,0) / (0,64)); measured pair span == single span.
  - Contraction rows per block: 660 = 5x128 + 20. K=128 chunks keep all
    16 SDMA engines and all PE rows busy; the 20-row tails of all blocks
    ship as one small transfer.
  - DMA consolidation: ONE 1.38 MB transfer per pair (x both orientations
    + identity gather), 2 operator transfers, 1 tail transfer, 1 output
    transfer per pair. ~15 DMAs total, ~10.8 MB/core (the roofline).
  - Precision: fp8 e3m4 in, bf16 out (rel_err 1.85e-2 < 2e-2).
"""

import numpy as np
import ml_dtypes

# architecture constants (match reference init_kwargs)
B = 8
C_OUT = 96
NK = 11
G = 4
C_IN = C_OUT * NK          # 1056
HOUT = WOUT = 56
HIN = WIN = 60
EP = 2                     # extra pad
N_CORES = 8
BPC = C_OUT // N_CORES     # blocks per core = 12
CPC = BPC * NK             # channels per core = 132
NPAIR = BPC // 2           # block pairs per core = 6
KROWS = NK * HIN           # 660 contraction rows per block (V/H)
KM = 128                   # main chunk rows
NJM = 5                    # main chunks (5 x 128 = 640)
KT = KROWS - NJM * KM      # tail chunk rows = 20
NFREE = B * WOUT           # 448 matmul free dim
N_WARM = 16                # PE warmup matmuls while first DMAs fly
OPW = NJM * 112 + 2 * 56   # operator cols per block = 672
XTW = 112 + 2 * NFREE      # tail cols per block = 1008

F8 = ml_dtypes.float8_e3m4

_CACHE = {}


def _build_vh_operators(w1, w2, pad_hv):
    """Dense V/H operators per block: (C_OUT, 660, 56) fp32 each.
    Row r = k*60 + spatial_in for channel c = co*11 + k."""
    w1r = np.asarray(w1, np.float32).reshape(G, C_IN)
    w2r = np.asarray(w2, np.float32).reshape(G, C_IN)
    pad = np.asarray(pad_hv, np.int64)            # (C_IN, 2G)
    opv = np.zeros((C_OUT, KROWS, 56), np.float32)
    oph = np.zeros((C_OUT, KROWS, 56), np.float32)
    c_all = np.arange(C_IN)
    co_all = c_all // NK
    k_all = c_all % NK
    pos = np.arange(HOUT)
    for g in range(G):
        win = pos[None, :] + EP + pad[:, g][:, None]        # (C_IN, 56)
        ok = (win >= 0) & (win < WIN)
        cc, oo = np.nonzero(ok)
        np.add.at(oph, (co_all[cc], k_all[cc] * HIN + win[cc, oo], oo), w1r[g, cc])
        hin = pos[None, :] + EP + pad[:, G + g][:, None]
        ok = (hin >= 0) & (hin < HIN)
        cc, oo = np.nonzero(ok)
        np.add.at(opv, (co_all[cc], k_all[cc] * HIN + hin[cc, oo], oo), w2r[g, cc])
    return opv, oph


def _identity_slots(w3, idx_identit):
    """Per block: up to 4 (k, coeff) identity terms, padded to 4 slots."""
    w3r = np.asarray(w3, np.float32).reshape(G, C_OUT)
    idx = np.asarray(idx_identit, np.int64)       # (C_OUT, G)
    k_sel = idx - np.arange(C_OUT)[:, None] * NK
    assert np.all((k_sel >= 0) & (k_sel < NK))
    u = np.zeros((C_OUT, NK), np.float32)
    for g in range(G):
        np.add.at(u, (np.arange(C_OUT), k_sel[:, g]), w3r[g])
    ks = np.zeros((C_OUT, 4), np.int64)
    cf = np.zeros((C_OUT, 4), np.float32)
    for co in range(C_OUT):
        nz = np.nonzero(u[co])[0]
        ks[co, :len(nz)] = nz
        cf[co, :len(nz)] = u[co, nz]
    return ks, cf


def _build_nc():
    import concourse.bacc as bacc
    import concourse.tile as tile
    import concourse.bass as bass
    import concourse.mybir as mybir
    from contextlib import ExitStack

    f32 = mybir.dt.float32
    f8 = mybir.dt.float8e3
    bf16 = mybir.dt.bfloat16

    nc = bacc.Bacc(None, target_bir_lowering=False)
    # x per pair: [p, slot, blk, chunk, n]; slot 0 = h-major (V) / identity
    # chunk0, slot 1 = w-major (H) / identity chunk1; chunk 5 is identity.
    xall_d = nc.declare_dram_parameter(
        "xall", [BPC, KM, 2, NJM + 1, NFREE], f8, isOutput=False)
    # operators per block: [p, bi, 672]: 5x112 V|H chunks then 2x56 identity
    opall_d = nc.declare_dram_parameter(
        "opall", [KM, BPC, OPW], f8, isOutput=False)
    # tails: [p(20), bi, 1008]: 112 V|H operator, then x tails (o=0, o=1)
    opxt_d = nc.declare_dram_parameter(
        "opxt", [KT, BPC, XTW], f8, isOutput=False)
    # output: per pair [120, 3, 448] (cols: blk_e VH | blk_o VH | identity)
    out_d = nc.declare_dram_parameter(
        "out", [NPAIR, 120, 3, NFREE], bf16, isOutput=True)

    with tile.TileContext(nc) as tc, ExitStack() as ctx:
        xpool = ctx.enter_context(tc.tile_pool(name="xp", bufs=1))
        oppool = ctx.enter_context(tc.tile_pool(name="opp", bufs=1))
        spool = ctx.enter_context(tc.tile_pool(name="stg", bufs=1))
        wpool = ctx.enter_context(tc.tile_pool(name="wp", bufs=1))
        psum_pool = ctx.enter_context(
            tc.tile_pool(name="psum", bufs=2, space=bass.MemorySpace.PSUM))
        wppool = ctx.enter_context(
            tc.tile_pool(name="wpp", bufs=1, space=bass.MemorySpace.PSUM))

        # ---- ring S (sync): tails+operators interleaved with per-block x;
        # ring A (scalar) carries only outputs ----
        op_t = oppool.tile([KM, BPC, OPW], f8, tag="opall")
        xt_t = oppool.tile([KT, BPC, XTW], f8, tag="opxt")
        nc.sync.dma_start(xt_t[:], opxt_d[:])
        x_ts = [None] * BPC

        def load_x(bi):
            x_t = xpool.tile([KM, 2, NJM + 1, NFREE], f8, tag=f"x{bi}",
                             name=f"x{bi}")
            nc.sync.dma_start(x_t[:], xall_d[bi])
            x_ts[bi] = x_t

        for q in range(NPAIR):
            nc.sync.dma_start(op_t[:, 2 * q:2 * q + 2],
                              opall_d[:, 2 * q:2 * q + 2])
            load_x(2 * q)
            load_x(2 * q + 1)

        # ---- PE warmup on memset tiles (no DMA dependency) ----
        warm = wpool.tile([KM, NFREE], f8, tag="warm")
        wst = wpool.tile([KM, 56], f8, tag="wst")
        nc.vector.memset(warm[:], 0)
        nc.vector.memset(wst[:], 0)
        pw = wppool.tile([128, NFREE], f32, tag="pw")
        for w in range(N_WARM):
            pos = (0, 0) if w % 2 == 0 else (0, 64)
            dst = pw[0:56] if w % 2 == 0 else pw[64:120]
            nc.tensor.matmul(dst, wst[:], warm[:], start=True, stop=True,
                             tile_position=pos)

        # ---- main: 6 block pairs ----
        out_stgs = []
        for q in range(NPAIR):
            pvh = [psum_pool.tile([128, NFREE], f32, tag="pe", name=f"pe{q}"),
                   psum_pool.tile([128, NFREE], f32, tag="po", name=f"po{q}")]
            pi = psum_pool.tile([128, NFREE], f32, tag="pi", name=f"pi{q}")
            for b in (0, 1):
                pt = pvh[b]
                bi = 2 * q + b
                x_t = x_ts[bi]
                for j in range(NJM):
                    nc.tensor.matmul(pt[0:56], op_t[:, bi, j * 112:j * 112 + 56],
                                     x_t[:, 0, j, :],
                                     start=(j == 0), stop=False,
                                     tile_position=(0, 0))
                    nc.tensor.matmul(pt[64:120],
                                     op_t[:, bi, j * 112 + 56:(j + 1) * 112],
                                     x_t[:, 1, j, :],
                                     start=(j == 0), stop=False,
                                     tile_position=(0, 64))
                nc.tensor.matmul(pt[0:56], xt_t[:, bi, 0:56],
                                 xt_t[:, bi, 112:112 + NFREE],
                                 start=False, stop=True,
                                 tile_position=(0, 0))
                nc.tensor.matmul(pt[64:120], xt_t[:, bi, 56:112],
                                 xt_t[:, bi, 112 + NFREE:XTW],
                                 start=False, stop=True,
                                 tile_position=(0, 64))
            for c in range(2):
                kk = 128 if c == 0 else 96  # identity rows: 224 = 128 + 96
                o0 = NJM * 112 + c * 56
                nc.tensor.matmul(pi[0:56], op_t[0:kk, 2 * q, o0:o0 + 56],
                                 x_ts[2 * q][0:kk, c, NJM, :],
                                 start=(c == 0), stop=(c == 1),
                                 tile_position=(0, 0))
                nc.tensor.matmul(pi[64:120], op_t[0:kk, 2 * q + 1, o0:o0 + 56],
                                 x_ts[2 * q + 1][0:kk, c, NJM, :],
                                 start=(c == 0), stop=(c == 1),
                                 tile_position=(0, 64))
            # drain psums -> bf16 staging -> one DMA per pair (ring A)
            stg = spool.tile([120, 3, NFREE], bf16, tag=f"stg{q}", name=f"stg{q}")
            nc.scalar.copy(stg[:, 0, :], pvh[0][0:120])
            nc.vector.tensor_copy(stg[:, 1, :], pvh[1][0:120])
            nc.vector.tensor_copy(stg[0:56, 2, :], pi[0:56])
            nc.scalar.copy(stg[64:120, 2, :], pi[64:120])
            out_stgs.append(stg)
        for q, stg in enumerate(out_stgs):
            if q == NPAIR - 1:
                nc.sync.dma_start(out_d[q, :, 0:2], stg[:, 0:2, :])
                nc.sync.dma_start(out_d[q, :, 2], stg[:, 2, :])
            else:
                nc.sync.dma_start(out_d[q], stg[:])
    nc.finalize()
    return nc


def prepare_inputs(x, w1, w2, w3, pad_hv, idx_identit):
    """Host-side shard prep. Returns in_maps (list of 8 dicts)."""
    x = np.asarray(x)
    xq = x.astype(F8)                                     # (B, C, 60, 60)
    opv, oph = _build_vh_operators(w1, w2, pad_hv)        # (96, 660, 56) f32
    ks, cf = _identity_slots(w3, idx_identit)             # (96,4) each
    eye = np.eye(56, dtype=np.float32)

    in_maps = []
    for i in range(N_CORES):
        blocks = np.arange(i * BPC, (i + 1) * BPC)
        csl = slice(i * CPC, (i + 1) * CPC)
        # h-major rows (k,h'), cols (b,w); w-major rows (k,w'), cols (b,h)
        ch = xq[:, csl, :, EP:EP + WOUT]                   # (8, 132, 60, 56)
        ch = ch.transpose(1, 2, 0, 3).reshape(BPC, KROWS, NFREE)
        cw = xq[:, csl, EP:EP + HOUT, :]                   # (8, 132, 56, 60)
        cw = cw.transpose(1, 3, 0, 2).reshape(BPC, KROWS, NFREE)
        xo = np.stack([ch, cw], axis=1)                    # (12, 2, 660, 448)
        # identity gather rows (slot, h'): 224 per block -> chunks 128 + 96
        chan = (blocks[:, None] * NK + ks[blocks]).astype(np.int64)  # (12,4)
        xi_raw = xq[:, chan, EP:EP + HOUT, EP:EP + WOUT]   # (8, 12, 4, 56, 56)
        xi_raw = xi_raw.transpose(1, 2, 3, 0, 4).reshape(BPC, 224, NFREE)
        # xall: [bi, p, slot, chunk(6), n]
        xall = np.zeros((BPC, KM, 2, NJM + 1, NFREE), F8)
        main = xo[:, :, :NJM * KM].reshape(BPC, 2, NJM, KM, NFREE)
        xall[:, :, :, :NJM] = main.transpose(0, 3, 1, 2, 4)
        xi_chunks = np.zeros((BPC, 2, KM, NFREE), F8)
        xi_chunks[:, 0] = xi_raw[:, :KM]
        xi_chunks[:, 1, :224 - KM] = xi_raw[:, KM:]
        xall[:, :, :, NJM] = xi_chunks.transpose(0, 2, 1, 3)
        # opall: [p, bi, 672]
        opvh = np.concatenate([opv[blocks], oph[blocks]], axis=2)  # (12,660,112)
        opm = (opvh[:, :NJM * KM].reshape(BPC, NJM, KM, 112)
               .transpose(2, 0, 1, 3).reshape(KM, BPC, NJM * 112))
        bands = (cf[blocks][:, :, None, None] * eye).reshape(BPC, 224, 56)
        opi_pad = np.zeros((BPC, 2, KM, 56), np.float32)
        opi_pad[:, 0] = bands[:, :KM]
        opi_pad[:, 1, :224 - KM] = bands[:, KM:]
        opi = opi_pad.transpose(2, 0, 1, 3).reshape(KM, BPC, 112)
        opall = np.ascontiguousarray(
            np.concatenate([opm, opi], axis=2)).astype(F8)  # (128, 12, 672)
        # opxt: [p(20), bi, 1008]: operator tail 112 | x tail o=0 | o=1
        optail = opvh[:, NJM * KM:].transpose(1, 0, 2)      # (20, 12, 112)
        xtail = (xo[:, :, NJM * KM:]                        # (12, 2, 20, 448)
                 .transpose(2, 0, 1, 3).reshape(KT, BPC, 2 * NFREE))
        opxt = np.ascontiguousarray(
            np.concatenate([optail, xtail.reshape(KT, BPC, 2 * NFREE)], axis=2)
        ).astype(F8)                                        # (20, 12, 1008)
        in_maps.append({"xall": xall, "opall": opall, "opxt": opxt})
    return in_maps


def unshard(results):
    """-> (out_h, out_v, out_i) each (B, C_OUT, 56, 56) fp32."""
    o = np.stack([np.asarray(r["out"], np.float32) for r in results])
    # o: (8, NPAIR, 120, 3, 448); col 0 = blk_e, 1 = blk_o, 2 = identity
    vh = o[:, :, :, 0:2].transpose(0, 1, 3, 2, 4)  # (8, 6, 2, 120, 448)
    vh = vh.reshape(N_CORES, BPC, 120, NFREE)
    V = vh[:, :, 0:56].reshape(N_CORES, BPC, 56, B, WOUT)
    out_v = V.transpose(3, 0, 1, 2, 4).reshape(B, C_OUT, HOUT, WOUT)
    Hh = vh[:, :, 64:120].reshape(N_CORES, BPC, 56, B, HOUT)  # [.., w, b, h]
    out_h = Hh.transpose(3, 0, 1, 4, 2).reshape(B, C_OUT, HOUT, WOUT)
    ii = o[:, :, :, 2]                             # (8, 6, 120, 448)
    Ie = ii[:, :, 0:56].reshape(N_CORES, NPAIR, 56, B, WOUT)
    Io = ii[:, :, 64:120].reshape(N_CORES, NPAIR, 56, B, WOUT)
    I2 = np.stack([Ie, Io], axis=2)                # [core, pair, half, h, b, w]
    out_i = I2.transpose(4, 0, 1, 2, 3, 5).reshape(B, C_OUT, HOUT, WOUT)
    return out_h, out_v, out_i


def kernel(x, w1, w2, w3, pad_hv, idx_identit, b=B, hout=HOUT, wout=WOUT):
    from concourse.bass_utils import run_bass_kernel_spmd

    assert int(b) == B and int(hout) == HOUT and int(wout) == WOUT
    assert tuple(np.asarray(x).shape) == (B, C_IN, HIN, WIN)

    in_maps = prepare_inputs(x, w1, w2, w3, pad_hv, idx_identit)
    nc = _CACHE.get("nc")
    if nc is None:
        nc = _build_nc()
        _CACHE["nc"] = nc
    res = run_bass_kernel_spmd(nc, in_maps, core_ids=list(range(N_CORES)))
    return unshard(res.results)
